# revision 4
# baseline (speedup 1.0000x reference)
"""Trainium2 Bass kernel: per-pixel 19x19 batch blur (KPN-style).

Reference computation (see problem):
    out[b,c,i,j] = (1/361) * sum_{ki,kj} pad[b,c,i+ki,j+kj] * kern[b, i*W+j, ki, kj]
with `pad` the 9-pixel reflection-padded input, shapes:
    input  (2, 3, 256, 256) f32
    kernel (2, 65536, 19, 19) f32    <- 189 MB, dominates memory traffic

Sharding: pure data parallel over (batch, H-tile): 8 cores = 2 batches x 4
tiles of 64 output rows each.  Each core receives
  - its contiguous kernel slice  (16384, 361) f32  (23.7 MB)
  - its reflection-padded, host-transposed input slice (3, 274, 82) f32
    (82 rows = 64 + 2*9 halo, transposed so that *columns* land on SBUF
    partitions)
and writes a (256, 192) f32 block = [jblk*128+j, c*64+i] that the host
transposes back into (3, 64, 256).

Device algorithm (per core), pixels-on-partitions layout:
  1. DMA padT -> 3 SBUF tiles [128 cols x (3 ch * 82 rows)] (+ an 18-col tile).
  2. ScalarE builds SHIFTC[c][jblk]: [128 parts (col j) x (kj:19, row:82)]
     = pad[row, jblk*128 + j + kj] -- 19 column-shifted replicas of the
     (tiny) padded image.  This makes the im2col patch of output pixel
     (i, j) a pure strided AP view:  patch(ki,kj) = SHIFTC[j, kj*82 + i+ki].
  3. Kernel slice streamed in 8 chunks [128 parts (j) x (16 rows * 361)].
  4. For each (row i, jblk, channel c): one fused VectorE
     tensor_tensor_reduce:
        prod = kern_tile[j, i_local, :] * SHIFTC_view(i)   (elementwise, 361)
        out[j] = sum(prod) / 361
     accumulated straight into the per-core output tile column (c*64+i).
  5. DMA the two [128 x 192] output tiles to DRAM.

All f32 (TTR runs at 1x DVE rate regardless of dtype for these strided APs).
"""

import os
import sys

import numpy as np

for _p in ("/opt/trn_rl_repo", os.path.expanduser("~/.axon_site/_ro/trn_rl_repo")):
    if os.path.isdir(_p) and _p not in sys.path:
        sys.path.insert(0, _p)

from contextlib import ExitStack

from concourse import bacc, bass_utils, mybir, tile
from concourse.ap import AP

# Problem constants (hardcoded per the self-containment contract).
B, C, H, W = 2, 3, 256, 256
L = 19
PAD = L // 2  # 9
K2 = L * L  # 361
N_CORES = 8
ROWS_PER_CORE = H // 4  # 64  (4 H-tiles x 2 batches = 8 cores)
PR = ROWS_PER_CORE + 2 * PAD  # 82 padded rows per core
PW = W + 2 * PAD  # 274 padded cols
R_CHUNK = 16  # output rows per kernel-DMA chunk
N_IBLK = ROWS_PER_CORE // R_CHUNK  # 4
F32 = mybir.dt.float32

_CACHE: dict = {}


def _build_program():
    nc = bacc.Bacc(
        "TRN2",
        target_bir_lowering=False,
        debug=False,
        enable_asserts=False,
        num_devices=N_CORES,
    )
    kern = nc.dram_tensor("kern", [ROWS_PER_CORE * W, K2], F32, kind="ExternalInput")
    padt = nc.dram_tensor("padt", [C, PW, PR], F32, kind="ExternalInput")
    outd = nc.dram_tensor("out", [2 * 128, C * ROWS_PER_CORE], F32, kind="ExternalOutput")

    mult = mybir.AluOpType.mult
    add = mybir.AluOpType.add

    with tile.TileContext(nc) as tc, ExitStack() as ctx:
        cpool = ctx.enter_context(tc.tile_pool(name="cpool", bufs=1))
        kpool = ctx.enter_context(tc.tile_pool(name="kpool", bufs=3))
        spool = ctx.enter_context(tc.tile_pool(name="spool", bufs=2))

        # --- padded-transposed image: 3 column-tiles (128, 128, 18 cols) ---
        col_tiles = []  # [ (tile_ap, col0, ncols) ]
        for t, (c0, ncols) in enumerate([(0, 128), (128, 128), (256, PW - 256)]):
            pt = cpool.tile([ncols, C * PR], F32, name=f"padcol{t}")
            nc.sync.dma_start(
                out=AP(pt.tensor, 0, [(C * PR, ncols), (PR, C), (1, PR)]),
                in_=AP(padt, c0 * PR, [(PR, ncols), (PW * PR, C), (1, PR)]),
            )
            col_tiles.append(pt)

        # --- shift matrix D [128 x 328]: D[p, p + 164] = 1, else 0.
        # lhsT = D[:, 164 + s : 164 + s + 128] is the shift-by-s matrix:
        #   (lhsT.T @ rhs)[j, :] = rhs[j + s, :]   (zero rows where j+s is
        # out of range), so a column shift crossing a col-tile boundary is
        # two PSUM-accumulated matmuls.  Engines can't read SBUF at an
        # unaligned partition base; TensorE contraction does the cross-
        # partition move instead.
        dmat = cpool.tile([128, 328], F32, name="dmat")
        nc.gpsimd.memset(dmat[:, :], 0.0)
        nc.gpsimd.affine_select(
            out=dmat[:, :],
            in_=dmat[:, :],
            compare_op=mybir.AluOpType.not_equal,
            fill=1.0,
            base=164,
            pattern=[[-1, 328]],
            channel_multiplier=1,
        )

        # --- SHIFTC[c][jblk]: [128 x (19*82)];  [j, kj*82 + r] = pad[c, r, jb*128+j+kj]
        ppool = ctx.enter_context(tc.tile_pool(name="ppool", bufs=4, space="PSUM"))
        shiftc = {}
        for jb in range(2):
            for c in range(3):
                sc = cpool.tile([128, L * PR], F32, name=f"shiftc_{c}_{jb}")
                shiftc[(c, jb)] = sc
                for kj in range(L):
                    ps = ppool.tile([128, PR], F32, name="ps", tag="ps")
                    # piece 1: j in [0, 128-kj): from col tile jb, parts j+kj
                    nc.tensor.matmul(
                        ps[:, :],
                        dmat[:, 164 + kj : 164 + kj + 128],
                        col_tiles[jb][:, c * PR : (c + 1) * PR],
                        start=True,
                        stop=(kj == 0),
                    )
                    if kj:
                        # piece 2: j in [128-kj, 128): parts j+kj-128 of tile jb+1
                        nparts2 = col_tiles[jb + 1].shape[0]
                        nc.tensor.matmul(
                            ps[:, :],
                            AP(dmat.tensor, 36 + kj, [(328, nparts2), (1, 128)]),
                            col_tiles[jb + 1][:, c * PR : (c + 1) * PR],
                            start=False,
                            stop=True,
                        )
                    nc.scalar.copy(out=sc[:, kj * PR : (kj + 1) * PR], in_=ps[:, :])

        # --- output tiles [128 x (3*64)] per jblk ---
        outt = [cpool.tile([128, C * ROWS_PER_CORE], F32, name=f"outt{jb}") for jb in range(2)]

        # --- main loop: stream kernel chunks, fused multiply-reduce per row ---
        kfree = R_CHUNK * K2
        for iblk in range(N_IBLK):
            for jb in range(2):
                kt = kpool.tile([128, kfree], F32, name="kt", tag="kt")
                base = (iblk * R_CHUNK * W + jb * 128) * K2
                nc.sync.dma_start(
                    out=AP(kt.tensor, 0, [(kfree, 128), (K2, R_CHUNK), (1, K2)]),
                    in_=AP(kern, base, [(K2, 128), (W * K2, R_CHUNK), (1, K2)]),
                )
                for c in range(3):
                    for il in range(R_CHUNK):
                        i = iblk * R_CHUNK + il
                        scr = spool.tile([128, K2], F32, name="scr", tag="scr")
                        col = c * ROWS_PER_CORE + i
                        # out = (in0 * 1/361) * in1 ; accum_out = sum(out).
                        # (The fused ant-DVE tensor_tensor_reduce faults on
                        # this runtime; InstTensorScalarPtr is standard ISA.)
                        nc.vector.scalar_tensor_tensor(
                            out=AP(scr.tensor, 0, [(K2, 128), (L, L), (1, L)]),
                            in0=AP(kt.tensor, il * K2, [(kfree, 128), (L, L), (1, L)]),
                            scalar=1.0 / K2,
                            in1=AP(
                                shiftc[(c, jb)].tensor,
                                i,
                                [(L * PR, 128), (1, L), (PR, L)],
                            ),
                            op0=mult,
                            op1=mult,
                            accum_out=outt[jb][:, col : col + 1],
                        )

        for jb in range(2):
            nc.sync.dma_start(
                out=AP(outd, jb * 128 * (C * ROWS_PER_CORE), [(C * ROWS_PER_CORE, 128), (1, C * ROWS_PER_CORE)]),
                in_=outt[jb][:, :],
            )

    nc.compile()
    return nc


def _program():
    if "nc" not in _CACHE:
        _CACHE["nc"] = _build_program()
    return _CACHE["nc"]


def _shard_inputs(input, kernel):
    inp = np.ascontiguousarray(np.asarray(input, dtype=np.float32))
    kern = np.asarray(kernel, dtype=np.float32)
    pad = np.pad(inp, ((0, 0), (0, 0), (PAD, PAD), (PAD, PAD)), mode="reflect")
    in_maps = []
    for core in range(N_CORES):
        b, q = divmod(core, 4)
        sl = pad[b, :, q * ROWS_PER_CORE : q * ROWS_PER_CORE + PR, :]  # (3, 82, 274)
        padt = np.ascontiguousarray(sl.transpose(0, 2, 1))  # (3, 274, 82)
        ks = np.ascontiguousarray(
            kern[b].reshape(H * W, K2)[q * ROWS_PER_CORE * W : (q + 1) * ROWS_PER_CORE * W]
        )
        in_maps.append({"kern": ks, "padt": padt})
    return in_maps


def _unshard_output(results):
    out = np.empty((B, C, H, W), dtype=np.float32)
    for core in range(N_CORES):
        b, q = divmod(core, 4)
        arr = np.asarray(results[core]["out"])  # (256, 192) = [jb*128+j, c*64+i]
        blk = arr.reshape(2, 128, C, ROWS_PER_CORE).transpose(2, 3, 0, 1)
        out[b, :, q * ROWS_PER_CORE : (q + 1) * ROWS_PER_CORE, :] = blk.reshape(
            C, ROWS_PER_CORE, W
        )
    return out


def run_sharded(inputs, **kw):
    """Run the compiled SPMD program; returns BassKernelResults (for profiling)."""
    in_maps = _shard_inputs(inputs["input"], inputs["kernel"])
    return bass_utils.run_bass_kernel_spmd(
        _program(), in_maps, core_ids=list(range(N_CORES)), **kw
    )


def kernel(input, kernel):
    res = run_sharded({"input": input, "kernel": kernel})
    return _unshard_output(res.results)


# revision 5
# speedup vs baseline: 1.6747x; 1.6747x over previous
"""Trainium2 Bass kernel: per-pixel 19x19 batch blur (KPN-style).

Reference computation:
    out[b,c,i,j] = (1/361) * sum_{ki,kj} pad[b,c,i+ki,j+kj] * kern[b, i*W+j, ki, kj]
with `pad` the 9-pixel reflection-padded input, shapes:
    input  (2, 3, 256, 256) f32
    kernel (2, 65536, 19, 19) f32    <- 189 MB, dominates memory traffic

Sharding: pure data parallel over (batch, H-tile): 8 cores = 2 batches x 4
tiles of 64 output rows each.  Each core receives
  - its contiguous kernel slice  (16384, 361) f32  (23.7 MB)
  - SHIFTC [2, 3, 128, 19*82] f32 (4.8 MB): for jblk/channel, partition j
    holds the flattened sliding strips  SHIFTC[jb,c,j, 19*r+kj] =
    pad[c, r, jb*128+j+kj].  This is the im2col halo prep done host-side
    (19x replication of the 0.27 MB padded slice); with this layout the
    361-tap patch of output row i is CONTIGUOUS at free offset 19*i,
    because k2 = 19*ki + kj.
and writes a (256, 192) f32 block = [jblk*128+j, c*64+i] that the host
transposes back into (3, 64, 256).

Device algorithm (per core), pixels-on-partitions:
  - kernel slice streamed in chunks [128 parts (j) x (R_CHUNK rows * 361)]
  - per (row i, jblk, channel c): ONE fused VectorE scalar_tensor_tensor:
        out  = (kern_row * 1/361) * patch_view      (361 contiguous f32)
        accum_out[j] = sum(out)                     -> output column
    (the ant-custom tensor_tensor_reduce faults on this runtime;
     InstTensorScalarPtr with accum output is standard ISA and works)
"""

import os
import sys

import numpy as np

for _p in ("/opt/trn_rl_repo", os.path.expanduser("~/.axon_site/_ro/trn_rl_repo")):
    if os.path.isdir(_p) and _p not in sys.path:
        sys.path.insert(0, _p)

from contextlib import ExitStack

from concourse import bacc, bass_utils, mybir, tile
from concourse.ap import AP

# Problem constants (hardcoded per the self-containment contract).
B, C, H, W = 2, 3, 256, 256
L = 19
PAD = L // 2  # 9
K2 = L * L  # 361
N_CORES = 8
ROWS_PER_CORE = H // 4  # 64  (4 H-tiles x 2 batches = 8 cores)
PR = ROWS_PER_CORE + 2 * PAD  # 82 padded rows per core
SFREE = L * PR  # 1558 free elems per SHIFTC partition
R_CHUNK = 16  # output rows per kernel-DMA chunk
N_IBLK = ROWS_PER_CORE // R_CHUNK  # 4
F32 = mybir.dt.float32

_CACHE: dict = {}


def _build_program():
    nc = bacc.Bacc(
        "TRN2",
        target_bir_lowering=False,
        debug=False,
        enable_asserts=False,
        num_devices=N_CORES,
    )
    kern = nc.dram_tensor("kern", [ROWS_PER_CORE * W, K2], F32, kind="ExternalInput")
    shiftd = nc.dram_tensor("shiftc", [2, C, 128, SFREE], F32, kind="ExternalInput")
    outd = nc.dram_tensor("out", [2 * 128, C * ROWS_PER_CORE], F32, kind="ExternalOutput")

    mult = mybir.AluOpType.mult

    with tile.TileContext(nc) as tc, ExitStack() as ctx:
        cpool = ctx.enter_context(tc.tile_pool(name="cpool", bufs=1))
        kpool = ctx.enter_context(tc.tile_pool(name="kpool", bufs=3))
        spool = ctx.enter_context(tc.tile_pool(name="spool", bufs=2))

        # SHIFTC tiles, DMA'd in first-needed-first order (jb inner loop order
        # below is (jb, c); first STTs touch (0,c0) then (0,c1)...).
        shiftc = {}
        for jb in range(2):
            for c in range(C):
                sc = cpool.tile([128, SFREE], F32, name=f"shiftc_{c}_{jb}")
                shiftc[(c, jb)] = sc
                nc.sync.dma_start(
                    out=sc[:, :],
                    in_=AP(shiftd, (jb * C + c) * 128 * SFREE, [(SFREE, 128), (1, SFREE)]),
                )

        outt = [cpool.tile([128, C * ROWS_PER_CORE], F32, name=f"outt{jb}") for jb in range(2)]

        kfree = R_CHUNK * K2
        for iblk in range(N_IBLK):
            for jb in range(2):
                kt = kpool.tile([128, kfree], F32, name="kt", tag="kt")
                base = (iblk * R_CHUNK * W + jb * 128) * K2
                nc.sync.dma_start(
                    out=AP(kt.tensor, 0, [(kfree, 128), (K2, R_CHUNK), (1, K2)]),
                    in_=AP(kern, base, [(K2, 128), (W * K2, R_CHUNK), (1, K2)]),
                )
                for c in range(C):
                    for il in range(R_CHUNK):
                        i = iblk * R_CHUNK + il
                        scr = spool.tile([128, K2], F32, name="scr", tag="scr")
                        col = c * ROWS_PER_CORE + i
                        # out = (in0 * 1/361) * in1 ; accum_out = sum(out).
                        # All three APs are flat contiguous 361-elem runs.
                        nc.vector.scalar_tensor_tensor(
                            out=scr[:, :],
                            in0=AP(kt.tensor, il * K2, [(kfree, 128), (1, K2)]),
                            scalar=1.0 / K2,
                            in1=AP(shiftc[(c, jb)].tensor, i * L, [(SFREE, 128), (1, K2)]),
                            op0=mult,
                            op1=mult,
                            accum_out=outt[jb][:, col : col + 1],
                        )

        for jb in range(2):
            nc.sync.dma_start(
                out=AP(
                    outd,
                    jb * 128 * (C * ROWS_PER_CORE),
                    [(C * ROWS_PER_CORE, 128), (1, C * ROWS_PER_CORE)],
                ),
                in_=outt[jb][:, :],
            )

    nc.compile()
    return nc


def _program():
    if "nc" not in _CACHE:
        _CACHE["nc"] = _build_program()
    return _CACHE["nc"]


def _shard_inputs(input, kernel):
    inp = np.ascontiguousarray(np.asarray(input, dtype=np.float32))
    kern = np.asarray(kernel, dtype=np.float32)
    pad = np.pad(inp, ((0, 0), (0, 0), (PAD, PAD), (PAD, PAD)), mode="reflect")
    # sliding horizontal strips: strips[b, c, r, j, kj] = pad[b, c, r, j + kj]
    strips = np.lib.stride_tricks.sliding_window_view(pad, L, axis=3)
    in_maps = []
    for core in range(N_CORES):
        b, q = divmod(core, 4)
        r0 = q * ROWS_PER_CORE
        # SHIFTC[jb, c, j, 19*r + kj] = pad[b, c, r0 + r, jb*128 + j + kj]
        s = strips[b, :, r0 : r0 + PR, :, :]  # (C, PR, 256, L)
        s = s.transpose(2, 0, 1, 3).reshape(2, 128, C, PR * L)  # (jb*128+j, c, r*L+kj)
        sc = np.ascontiguousarray(s.transpose(0, 2, 1, 3))  # (2, C, 128, SFREE)
        ks = np.ascontiguousarray(
            kern[b].reshape(H * W, K2)[q * ROWS_PER_CORE * W : (q + 1) * ROWS_PER_CORE * W]
        )
        in_maps.append({"kern": ks, "shiftc": sc})
    return in_maps


def _unshard_output(results):
    out = np.empty((B, C, H, W), dtype=np.float32)
    for core in range(N_CORES):
        b, q = divmod(core, 4)
        arr = np.asarray(results[core]["out"])  # (256, 192) = [jb*128+j, c*64+i]
        blk = arr.reshape(2, 128, C, ROWS_PER_CORE).transpose(2, 3, 0, 1)
        out[b, :, q * ROWS_PER_CORE : (q + 1) * ROWS_PER_CORE, :] = blk.reshape(
            C, ROWS_PER_CORE, W
        )
    return out


def run_sharded(inputs, **kw):
    """Run the compiled SPMD program; returns BassKernelResults (for profiling)."""
    in_maps = _shard_inputs(inputs["input"], inputs["kernel"])
    return bass_utils.run_bass_kernel_spmd(
        _program(), in_maps, core_ids=list(range(N_CORES)), **kw
    )


def kernel(input, kernel):
    res = run_sharded({"input": input, "kernel": kernel})
    return _unshard_output(res.results)


# revision 7
# speedup vs baseline: 1.7519x; 1.0461x over previous
"""Trainium2 Bass kernel: per-pixel 19x19 batch blur (KPN-style).

Reference computation:
    out[b,c,i,j] = (1/361) * sum_{ki,kj} pad[b,c,i+ki,j+kj] * kern[b, i*W+j, ki, kj]
with `pad` the 9-pixel reflection-padded input, shapes:
    input  (2, 3, 256, 256) f32
    kernel (2, 65536, 19, 19) f32    <- 189 MB, dominates memory traffic

Sharding: pure data parallel over (batch, H-tile): 8 cores = 2 batches x 4
tiles of 64 output rows each.  Each core receives
  - its contiguous kernel slice  (16384, 361) f32  (23.7 MB)
  - SHIFTC [2, 3, 128, 19*82] f32 (4.8 MB): for jblk/channel, partition j
    holds the flattened sliding strips  SHIFTC[jb,c,j, 19*r+kj] =
    pad[c, r, jb*128+j+kj].  This is the im2col halo prep done host-side
    (19x replication of the 0.27 MB padded slice); with this layout the
    361-tap patch of output row i is CONTIGUOUS at free offset 19*i,
    because k2 = 19*ki + kj.
and writes a (256, 192) f32 block = [jblk*128+j, c*64+i] that the host
transposes back into (3, 64, 256).

Device algorithm (per core), pixels-on-partitions:
  - kernel slice streamed in chunks [128 parts (j) x (R_CHUNK rows * 361)]
  - per (row i, jblk, channel c): ONE fused VectorE scalar_tensor_tensor:
        out  = (kern_row * 1/361) * patch_view      (361 contiguous f32)
        accum_out[j] = sum(out)                     -> output column
    (the ant-custom tensor_tensor_reduce faults on this runtime;
     InstTensorScalarPtr with accum output is standard ISA and works)
"""

import os
import sys

import numpy as np

for _p in ("/opt/trn_rl_repo", os.path.expanduser("~/.axon_site/_ro/trn_rl_repo")):
    if os.path.isdir(_p) and _p not in sys.path:
        sys.path.insert(0, _p)

from contextlib import ExitStack

from concourse import bacc, bass_utils, mybir, tile
from concourse.ap import AP

# Problem constants (hardcoded per the self-containment contract).
B, C, H, W = 2, 3, 256, 256
L = 19
PAD = L // 2  # 9
K2 = L * L  # 361
N_CORES = 8
ROWS_PER_CORE = H // 4  # 64  (4 H-tiles x 2 batches = 8 cores)
PR = ROWS_PER_CORE + 2 * PAD  # 82 padded rows per core
SFREE = L * PR  # 1558 free elems per SHIFTC partition
R_CHUNK = 8  # output rows per kernel-DMA chunk
N_IBLK = ROWS_PER_CORE // R_CHUNK  # 8
F32 = mybir.dt.float32

_CACHE: dict = {}


def _build_program():
    nc = bacc.Bacc(
        "TRN2",
        target_bir_lowering=False,
        debug=False,
        enable_asserts=False,
        num_devices=N_CORES,
    )
    kern = nc.dram_tensor("kern", [ROWS_PER_CORE * W, K2], F32, kind="ExternalInput")
    shiftd = nc.dram_tensor("shiftc", [2, C, 128, SFREE], F32, kind="ExternalInput")
    outd = nc.dram_tensor("out", [2 * 128, C * ROWS_PER_CORE], F32, kind="ExternalOutput")

    mult = mybir.AluOpType.mult

    with tile.TileContext(nc) as tc, ExitStack() as ctx:
        cpool = ctx.enter_context(tc.tile_pool(name="cpool", bufs=1))
        kpool = ctx.enter_context(tc.tile_pool(name="kpool", bufs=3))
        spool = ctx.enter_context(tc.tile_pool(name="spool", bufs=2))

        # SHIFTC tiles: issue the first-needed one, then the first kernel
        # chunk, then the rest — so the first STT's inputs aren't queued
        # behind 4.8 MB of later SHIFTC traffic.
        shiftc = {}

        def _load_shiftc(c, jb):
            sc = cpool.tile([128, SFREE], F32, name=f"shiftc_{c}_{jb}")
            shiftc[(c, jb)] = sc
            nc.sync.dma_start(
                out=sc[:, :],
                in_=AP(shiftd, (jb * C + c) * 128 * SFREE, [(SFREE, 128), (1, SFREE)]),
            )

        kfree = R_CHUNK * K2

        def _load_chunk(iblk, jb):
            kt = kpool.tile([128, kfree], F32, name="kt", tag="kt")
            base = (iblk * R_CHUNK * W + jb * 128) * K2
            nc.sync.dma_start(
                out=AP(kt.tensor, 0, [(kfree, 128), (K2, R_CHUNK), (1, K2)]),
                in_=AP(kern, base, [(K2, 128), (W * K2, R_CHUNK), (1, K2)]),
            )
            return kt

        _load_shiftc(0, 0)
        kts = {(0, 0): _load_chunk(0, 0)}
        for c in range(1, C):
            _load_shiftc(c, 0)
        for c in range(C):
            _load_shiftc(c, 1)

        outt = [cpool.tile([128, C * ROWS_PER_CORE], F32, name=f"outt{jb}") for jb in range(2)]

        for iblk in range(N_IBLK):
            for jb in range(2):
                kt = kts.pop((iblk, jb), None)
                if kt is None:
                    kt = _load_chunk(iblk, jb)
                for c in range(C):
                    for il in range(R_CHUNK):
                        i = iblk * R_CHUNK + il
                        scr = spool.tile([128, K2], F32, name="scr", tag="scr")
                        col = c * ROWS_PER_CORE + i
                        # out = (in0 * 1/361) * in1 ; accum_out = sum(out).
                        # All three APs are flat contiguous 361-elem runs.
                        nc.vector.scalar_tensor_tensor(
                            out=scr[:, :],
                            in0=AP(kt.tensor, il * K2, [(kfree, 128), (1, K2)]),
                            scalar=1.0 / K2,
                            in1=AP(shiftc[(c, jb)].tensor, i * L, [(SFREE, 128), (1, K2)]),
                            op0=mult,
                            op1=mult,
                            accum_out=outt[jb][:, col : col + 1],
                        )

        for jb in range(2):
            nc.sync.dma_start(
                out=AP(
                    outd,
                    jb * 128 * (C * ROWS_PER_CORE),
                    [(C * ROWS_PER_CORE, 128), (1, C * ROWS_PER_CORE)],
                ),
                in_=outt[jb][:, :],
            )

    nc.compile()
    return nc


def _program():
    if "nc" not in _CACHE:
        _CACHE["nc"] = _build_program()
    return _CACHE["nc"]


def _shard_inputs(input, kernel):
    inp = np.ascontiguousarray(np.asarray(input, dtype=np.float32))
    kern = np.asarray(kernel, dtype=np.float32)
    pad = np.pad(inp, ((0, 0), (0, 0), (PAD, PAD), (PAD, PAD)), mode="reflect")
    # sliding horizontal strips: strips[b, c, r, j, kj] = pad[b, c, r, j + kj]
    strips = np.lib.stride_tricks.sliding_window_view(pad, L, axis=3)
    in_maps = []
    for core in range(N_CORES):
        b, q = divmod(core, 4)
        r0 = q * ROWS_PER_CORE
        # SHIFTC[jb, c, j, 19*r + kj] = pad[b, c, r0 + r, jb*128 + j + kj]
        s = strips[b, :, r0 : r0 + PR, :, :]  # (C, PR, 256, L)
        s = s.transpose(2, 0, 1, 3).reshape(2, 128, C, PR * L)  # (jb*128+j, c, r*L+kj)
        sc = np.ascontiguousarray(s.transpose(0, 2, 1, 3))  # (2, C, 128, SFREE)
        ks = np.ascontiguousarray(
            kern[b].reshape(H * W, K2)[q * ROWS_PER_CORE * W : (q + 1) * ROWS_PER_CORE * W]
        )
        in_maps.append({"kern": ks, "shiftc": sc})
    return in_maps


def _unshard_output(results):
    out = np.empty((B, C, H, W), dtype=np.float32)
    for core in range(N_CORES):
        b, q = divmod(core, 4)
        arr = np.asarray(results[core]["out"])  # (256, 192) = [jb*128+j, c*64+i]
        blk = arr.reshape(2, 128, C, ROWS_PER_CORE).transpose(2, 3, 0, 1)
        out[b, :, q * ROWS_PER_CORE : (q + 1) * ROWS_PER_CORE, :] = blk.reshape(
            C, ROWS_PER_CORE, W
        )
    return out


def run_sharded(inputs, **kw):
    """Run the compiled SPMD program; returns BassKernelResults (for profiling)."""
    in_maps = _shard_inputs(inputs["input"], inputs["kernel"])
    return bass_utils.run_bass_kernel_spmd(
        _program(), in_maps, core_ids=list(range(N_CORES)), **kw
    )


def kernel(input, kernel):
    res = run_sharded({"input": input, "kernel": kernel})
    return _unshard_output(res.results)
